# revision 1
# baseline (speedup 1.0000x reference)
"""Bundle-adjustment projection kernel for 8 Trainium2 NeuronCores.

out[v, n, :] = (u, v) pixel projection of point n under view v
(reference: nn_BundleAdjustmentModel).

Sharding: data-parallel over views — 8 views per core, points replicated.
Per core the pipeline is pure elementwise work spread across DVE / ACT /
GPSIMD engines (PE matmul loses badly here: K=4 contractions with fp32
need 4 cyc/row plus stationary churn):

  zc = R2.p - depth                  (fp32: ACT init + 2 DVE scalar_tensor_tensor)
  rs = clip(1/zc, +-1/eps)           (DVE reciprocal_approx_fast + GPSIMD clip,
                                      == sign(zc)/max(|zc|, eps))
  a  = (-f*R0.p - f*tx)/256          (fp16 chain, /256 keeps a*rs in fp16 range)
  b  = ( f*R1.p + f*ty)/256          (fp16 chain)
  u  = (a*rs)*256 + cx ; v = (b*rs)*256 + cy   (ACT, interleaved strided write)

Host precomputes the per-view 3x4 affine coefficient rows (folding focal/
softplus/sign), which is O(V) work; all O(V*N) work runs on device.
"""
import sys
import types

import numpy as np

V = 64
N = 500000
NC = 8  # cores
NV_LOC = V // NC  # views per core
TCOLS = 3908  # even (fp16 2x mode) and >= ceil(N/128); 128*3908 = 500224
NPAD = 128 * TCOLS
CHUNK = 1954
AB_SCALE = 256.0
MIN_FOCAL = 50.0
MIN_DISTANCE = 0.25
Z_EPS = 1e-4

_CACHE = {}


def _setup_paths():
    if "/opt/trn_rl_repo" not in sys.path:
        sys.path.insert(0, "/opt/trn_rl_repo")
    # the axon trace path imports antenv.axon_hooks; provide a stub if absent
    try:
        import antenv
        if not hasattr(antenv, "axon_hooks"):
            mod = types.ModuleType("antenv.axon_hooks")
            mod._hook = None
            mod.set_axon_ntff_profile_hook = lambda h: setattr(mod, "_hook", h)
            mod.get_axon_ntff_profile_hook = lambda: mod._hook
            sys.modules["antenv.axon_hooks"] = mod
            antenv.axon_hooks = mod
    except ImportError:
        pass


def _build_nc():
    import concourse.bacc as bacc
    import concourse.mybir as mybir
    from concourse import tile

    dt = mybir.dt
    AF = mybir.ActivationFunctionType
    ALU = mybir.AluOpType

    nc = bacc.Bacc("TRN2", target_bir_lowering=False, debug=False)
    PX = nc.dram_tensor("PX", [128, TCOLS], dt.float32, kind="ExternalInput")
    PY = nc.dram_tensor("PY", [128, TCOLS], dt.float32, kind="ExternalInput")
    PZ = nc.dram_tensor("PZ", [128, TCOLS], dt.float32, kind="ExternalInput")
    MB = nc.dram_tensor("MB", [128, 100], dt.float32, kind="ExternalInput")
    OUT = nc.dram_tensor(
        "OUT", [NV_LOC, 128, 2 * TCOLS], dt.float32, kind="ExternalOutput"
    )

    chunks = [(0, CHUNK), (CHUNK, TCOLS - CHUNK)]

    with tile.TileContext(nc) as tc:
        with (
            tc.tile_pool(name="pts", bufs=1) as ppool,
            tc.tile_pool(name="cst", bufs=1) as cpool,
            tc.tile_pool(name="wrk", bufs=2) as wp,
        ):
            xs = ppool.tile([128, TCOLS], dt.float32)
            ys = ppool.tile([128, TCOLS], dt.float32)
            zs = ppool.tile([128, TCOLS], dt.float32)
            x16 = ppool.tile([128, TCOLS], dt.float16)
            y16 = ppool.tile([128, TCOLS], dt.float16)
            z16 = ppool.tile([128, TCOLS], dt.float16)
            nc.sync.dma_start(out=xs[:], in_=PX.ap())
            nc.sync.dma_start(out=ys[:], in_=PY.ap())
            nc.sync.dma_start(out=zs[:], in_=PZ.ap())
            nc.vector.tensor_copy(x16[:], xs[:])
            nc.vector.tensor_copy(y16[:], ys[:])
            nc.vector.tensor_copy(z16[:], zs[:])
            mb = cpool.tile([128, 100], dt.float32)
            nc.sync.dma_start(out=mb[:], in_=MB.ap())

            def col(j):
                return mb[:, j:j + 1]

            cxv = col(96)
            cyv = col(97)
            zp = col(98)  # 0.0

            for v in range(NV_LOC):
                q = 12 * v
                ma0, ma1, ma2, ma3 = col(q), col(q + 1), col(q + 2), col(q + 3)
                mb0, mb1, mb2, mb3 = col(q + 4), col(q + 5), col(q + 6), col(q + 7)
                mz0, mz1, mz2, mz3 = col(q + 8), col(q + 9), col(q + 10), col(q + 11)
                for (c0, w) in chunks:
                    s = slice(c0, c0 + w)
                    zc = wp.tile([128, CHUNK], dt.float32, name="zc", tag="zc")[:, :w]
                    rs = wp.tile([128, CHUNK], dt.float32, name="rs", tag="rs")[:, :w]
                    r16 = wp.tile([128, CHUNK], dt.float16, name="r16",
                                  tag="r16")[:, :w]
                    ac = wp.tile([128, CHUNK], dt.float16, name="ac", tag="ac")[:, :w]
                    bc = wp.tile([128, CHUNK], dt.float16, name="bc", tag="bc")[:, :w]
                    t2 = wp.tile([128, CHUNK], dt.float16, name="t2", tag="t2")[:, :w]
                    t3 = wp.tile([128, CHUNK], dt.float16, name="t3", tag="t3")[:, :w]
                    t4 = wp.tile([128, CHUNK], dt.float16, name="t4", tag="t4")[:, :w]
                    t5 = wp.tile([128, CHUNK], dt.float16, name="t5", tag="t5")[:, :w]
                    uv = wp.tile([128, 2 * CHUNK], dt.float32, name="uv",
                                 tag="uv")[:, :2 * w]

                    # z chain (fp32): zc = z*Mz2 + Mz3 + x*Mz0 + y*Mz1
                    nc.scalar.activation(zc, zs[:, s], AF.Identity,
                                         scale=mz2, bias=mz3)
                    nc.vector.scalar_tensor_tensor(
                        zc, xs[:, s], mz0, zc, op0=ALU.mult, op1=ALU.add)
                    nc.vector.scalar_tensor_tensor(
                        zc, ys[:, s], mz1, zc, op0=ALU.mult, op1=ALU.add)
                    # safe reciprocal: 1/zc clipped to +-1/eps, cast to fp16
                    nc.vector.reciprocal_approx_fast(out=rs, in_=zc)
                    nc.gpsimd.tensor_scalar(
                        r16, rs, 1.0 / Z_EPS, -1.0 / Z_EPS, ALU.min, ALU.max)
                    # a chain (fp16 /256): ac = (x*ma0+ma3) + y*ma1 + z*ma2
                    nc.scalar.activation(ac, x16[:, s], AF.Identity,
                                         scale=ma0, bias=ma3)
                    nc.vector.tensor_scalar(
                        t2, y16[:, s], ma1, 0.0, ALU.mult, ALU.add)
                    nc.vector.tensor_scalar(
                        t3, z16[:, s], ma2, 0.0, ALU.mult, ALU.add)
                    nc.vector.tensor_tensor(ac, ac, t2, ALU.add)
                    nc.vector.tensor_tensor(ac, ac, t3, ALU.add)
                    # b chain (fp16 /256): bc = (y*mb1+mb3) + x*mb0 + z*mb2
                    nc.scalar.activation(bc, y16[:, s], AF.Identity,
                                         scale=mb1, bias=mb3)
                    nc.vector.tensor_scalar(
                        t4, x16[:, s], mb0, 0.0, ALU.mult, ALU.add)
                    nc.vector.tensor_scalar(
                        t5, z16[:, s], mb2, 0.0, ALU.mult, ALU.add)
                    nc.vector.tensor_tensor(bc, bc, t4, ALU.add)
                    nc.vector.tensor_tensor(bc, bc, t5, ALU.add)
                    # project (in-place) + interleave with *256 and +cx/+cy
                    nc.vector.tensor_tensor(t2, ac, r16, ALU.mult)
                    nc.vector.tensor_tensor(t4, bc, r16, ALU.mult)
                    uvv = uv.rearrange("p (n two) -> p two n", two=2)
                    nc.scalar.activation(uvv[:, 0, :], t2, AF.Identity,
                                         scale=AB_SCALE, bias=cxv)
                    nc.scalar.activation(uvv[:, 1, :], t4, AF.Identity,
                                         scale=AB_SCALE, bias=cyv)
                    nc.sync.dma_start(
                        out=OUT.ap()[v][:, 2 * c0:2 * (c0 + w)], in_=uv)
    nc.compile()
    return nc


def _host_precompute(points, euler, translation_xy, translation_depth_raw,
                     focal_raw, cx, cy):
    """Replicate the reference's O(V) math in fp32 numpy."""
    euler = np.asarray(euler, np.float32)
    c = np.cos(euler)
    s = np.sin(euler)
    cx_, cy_, cz_ = c[:, 0], c[:, 1], c[:, 2]
    sx_, sy_, sz_ = s[:, 0], s[:, 1], s[:, 2]
    one = np.ones_like(cx_)
    zero = np.zeros_like(cx_)
    rx = np.stack([
        np.stack([one, zero, zero], -1),
        np.stack([zero, cx_, -sx_], -1),
        np.stack([zero, sx_, cx_], -1)], -2).astype(np.float32)
    ry = np.stack([
        np.stack([cy_, zero, sy_], -1),
        np.stack([zero, one, zero], -1),
        np.stack([-sy_, zero, cy_], -1)], -2).astype(np.float32)
    rz = np.stack([
        np.stack([cz_, -sz_, zero], -1),
        np.stack([sz_, cz_, zero], -1),
        np.stack([zero, zero, one], -1)], -2).astype(np.float32)
    rot = np.matmul(np.matmul(rx, ry), rz).astype(np.float32)  # [V,3,3]

    tdr = np.asarray(translation_depth_raw, np.float32)
    depth = (np.logaddexp(tdr, np.float32(0.0)).astype(np.float32)
             + np.float32(MIN_DISTANCE)).astype(np.float32)
    fr = np.float32(np.asarray(focal_raw).reshape(-1)[0])
    focal = np.float32(np.logaddexp(fr, np.float32(0.0))) + np.float32(MIN_FOCAL)
    txy = np.asarray(translation_xy, np.float32)

    # per-view coefficient block: [Ma(4) | Mb(4) | Mz(4)]; a/b rows /256
    M = np.zeros((V, 12), np.float32)
    M[:, 0:3] = (-focal / AB_SCALE) * rot[:, 0, :]
    M[:, 3] = (-focal / AB_SCALE) * txy[:, 0]
    M[:, 4:7] = (focal / AB_SCALE) * rot[:, 1, :]
    M[:, 7] = (focal / AB_SCALE) * txy[:, 1]
    M[:, 8:11] = rot[:, 2, :]
    M[:, 11] = -depth
    return M, np.float32(cx), np.float32(cy)


def kernel(points, euler, translation_xy, translation_depth_raw, focal_raw,
           cx, cy, _trace=False):
    _setup_paths()
    from concourse.bass_utils import run_bass_kernel_spmd

    if "nc" not in _CACHE:
        _CACHE["nc"] = _build_nc()
    nc = _CACHE["nc"]

    points = np.ascontiguousarray(np.asarray(points, np.float32))
    M, cxf, cyf = _host_precompute(
        points, euler, translation_xy, translation_depth_raw, focal_raw, cx, cy)

    pts_pad = np.zeros((NPAD, 3), np.float32)
    pts_pad[:N] = points
    planes = pts_pad.reshape(128, TCOLS, 3)
    px = np.ascontiguousarray(planes[:, :, 0])
    py = np.ascontiguousarray(planes[:, :, 1])
    pz = np.ascontiguousarray(planes[:, :, 2])

    in_maps = []
    for c in range(NC):
        mbrow = np.zeros(100, np.float32)
        mbrow[:96] = M[c * NV_LOC:(c + 1) * NV_LOC].reshape(-1)
        mbrow[96] = cxf
        mbrow[97] = cyf
        mbt = np.ascontiguousarray(
            np.broadcast_to(mbrow, (128, 100)).astype(np.float32))
        in_maps.append({"PX": px, "PY": py, "PZ": pz, "MB": mbt})

    res = run_bass_kernel_spmd(nc, in_maps, list(range(NC)), trace=_trace)
    _CACHE["last_results"] = res

    out = np.empty((V, N, 2), np.float32)
    for c in range(NC):
        o = res.results[c]["OUT"]  # [NV_LOC, 128, 2*TCOLS]
        o = o.reshape(NV_LOC, NPAD, 2)
        out[c * NV_LOC:(c + 1) * NV_LOC] = o[:, :N, :]
    return out



# revision 3
# speedup vs baseline: 1.4880x; 1.4880x over previous
"""Bundle-adjustment projection kernel for 8 Trainium2 NeuronCores.

out[v, n, :] = (u, v) pixel projection of point n under view v
(reference: nn_BundleAdjustmentModel).

Sharding: points N split 8 ways (62500/core); every core computes all 64
views for its slice. On-chip layout: partition p = 64*g + v where g splits
the core's points into 2 halves of 31250 — so every elementwise op runs
128 partitions wide.

The affine work runs on the otherwise-idle PE (tensor engine): per 512-col
chunk, three matmuls with block stationaries [7, 128] compute

  a  = (-f*R0 + cx*R2).p + (-f*tx - cx*depth)    (fp16 in, fp32 PSUM)
  b  = ( f*R1 + cy*R2).p + ( f*ty - cy*depth)    (fp16 in, fp32 PSUM)
  zc =            R2.p  - depth                   (fp32: zc feeds a pole,
                                                   fp16 would break the
                                                   |zc|<1e-4 clip region)

using moving tiles [7, 512] whose rows are (x,y,z) of the g=0 point,
(x,y,z) of the g=1 point, and a constant 1 (bias row). Then u = a*rs,
v = b*rs with rs = clip(1/zc, +-1e4):

  DVE:    rs_raw = reciprocal_approx_fast(zc)     (PSUM -> SBUF)
  GPSIMD: rs = clip(rs_raw)                       (SBUF -> SBUF, no port
                                                   contention with 1x DVE)
  DVE:    u = a*rs -> uv[:, 0::2], v = b*rs -> uv[:, 1::2]
  DMA:    uv [128, 1024] fp32 -> HBM (4 KiB/partition contiguous)

cx/cy are folded into the PE coefficients (u = (a + cx*zc)/zc = a/zc + cx
exactly when unclipped; error <= cx*(1-|zc|*1e4) <= 640 ~ 1.6e-4 of scale
on clipped points). Host does all O(V) coefficient math + O(N) transposes.
"""
import sys
import types

import numpy as np

V = 64
N = 500000
NC = 8  # cores
N_LOC = N // NC  # 62500 points per core
HALF = N_LOC // 2  # 31250 per partition-half
FW = 512  # chunk width (1 PSUM bank)
NCH = (HALF + FW - 1) // FW  # 62 chunks
F_PAD = NCH * FW  # 31744
Z_EPS = 1e-4
RS_MAX = 1.0 / Z_EPS
MIN_FOCAL = 50.0
MIN_DISTANCE = 0.25

_CACHE = {}


def _setup_paths():
    if "/opt/trn_rl_repo" not in sys.path:
        sys.path.insert(0, "/opt/trn_rl_repo")
    # the axon trace path imports antenv.axon_hooks; provide a stub if absent
    try:
        import antenv
        if not hasattr(antenv, "axon_hooks"):
            mod = types.ModuleType("antenv.axon_hooks")
            mod._hook = None
            mod.set_axon_ntff_profile_hook = lambda h: setattr(mod, "_hook", h)
            mod.get_axon_ntff_profile_hook = lambda: mod._hook
            sys.modules["antenv.axon_hooks"] = mod
            antenv.axon_hooks = mod
    except ImportError:
        pass


def _build_nc():
    import concourse.bacc as bacc
    import concourse.mybir as mybir
    from concourse import tile

    dt = mybir.dt
    ALU = mybir.AluOpType

    nc = bacc.Bacc("TRN2", target_bir_lowering=False, debug=False)
    MOV16 = nc.dram_tensor("MOV16", [7, F_PAD], dt.float16, kind="ExternalInput")
    MOV32 = nc.dram_tensor("MOV32", [7, F_PAD], dt.float32, kind="ExternalInput")
    STA = nc.dram_tensor("STA", [7, 256], dt.float16, kind="ExternalInput")
    STZ = nc.dram_tensor("STZ", [7, 128], dt.float32, kind="ExternalInput")
    OUT = nc.dram_tensor("OUT", [128, 2 * F_PAD], dt.float32, kind="ExternalOutput")

    with tile.TileContext(nc) as tc:
        with (
            tc.tile_pool(name="cst", bufs=1) as cpool,
            tc.tile_pool(name="wrk", bufs=3) as wp,
            tc.tile_pool(name="ps", bufs=2, space="PSUM") as pp,
        ):
            mov16 = cpool.tile([7, F_PAD], dt.float16)
            sta = cpool.tile([7, 256], dt.float16)
            stz = cpool.tile([7, 128], dt.float32)
            nc.sync.dma_start(out=mov16[:], in_=MOV16.ap())
            nc.sync.dma_start(out=sta[:], in_=STA.ap())
            nc.sync.dma_start(out=stz[:], in_=STZ.ap())

            for c in range(NCH):
                s = slice(c * FW, (c + 1) * FW)
                m32 = wp.tile([7, FW], dt.float32, name="m32", tag="m32")
                nc.sync.dma_start(out=m32[:], in_=MOV32.ap()[:, s])
                pa = pp.tile([128, FW], dt.float32, name="pa", tag="pa")
                pb = pp.tile([128, FW], dt.float32, name="pb", tag="pb")
                pz = pp.tile([128, FW], dt.float32, name="pz", tag="pz")
                nc.tensor.matmul(pz[:], stz[:], m32[:], start=True, stop=True)
                nc.tensor.matmul(pa[:], sta[:, 0:128], mov16[:, s],
                                 start=True, stop=True)
                nc.tensor.matmul(pb[:], sta[:, 128:256], mov16[:, s],
                                 start=True, stop=True)
                rs = wp.tile([128, FW], dt.float32, name="rs", tag="rs")
                rc = wp.tile([128, FW], dt.float32, name="rc", tag="rc")
                uv = wp.tile([128, 2 * FW], dt.float32, name="uv", tag="uv")
                nc.vector.reciprocal_approx_fast(out=rs[:], in_=pz[:])
                nc.gpsimd.tensor_scalar(rc[:], rs[:], RS_MAX, -RS_MAX,
                                        ALU.min, ALU.max)
                uvv = uv.rearrange("p (n two) -> p two n", two=2)
                nc.vector.tensor_tensor(uvv[:, 0, :], pa[:], rc[:], ALU.mult)
                nc.vector.tensor_tensor(uvv[:, 1, :], pb[:], rc[:], ALU.mult)
                nc.sync.dma_start(out=OUT.ap()[:, 2 * c * FW:2 * (c + 1) * FW],
                                  in_=uv)
    nc.compile()
    return nc


def _host_precompute(euler, translation_xy, translation_depth_raw, focal_raw,
                     cx, cy):
    """Per-view coefficient rows (fp32): Ca/sA, Cb/sB, Cz/sZ."""
    euler = np.asarray(euler, np.float32)
    c = np.cos(euler)
    s = np.sin(euler)
    cx_, cy_, cz_ = c[:, 0], c[:, 1], c[:, 2]
    sx_, sy_, sz_ = s[:, 0], s[:, 1], s[:, 2]
    one = np.ones_like(cx_)
    zero = np.zeros_like(cx_)
    rx = np.stack([
        np.stack([one, zero, zero], -1),
        np.stack([zero, cx_, -sx_], -1),
        np.stack([zero, sx_, cx_], -1)], -2).astype(np.float32)
    ry = np.stack([
        np.stack([cy_, zero, sy_], -1),
        np.stack([zero, one, zero], -1),
        np.stack([-sy_, zero, cy_], -1)], -2).astype(np.float32)
    rz = np.stack([
        np.stack([cz_, -sz_, zero], -1),
        np.stack([sz_, cz_, zero], -1),
        np.stack([zero, zero, one], -1)], -2).astype(np.float32)
    rot = np.matmul(np.matmul(rx, ry), rz).astype(np.float32)  # [V,3,3]

    tdr = np.asarray(translation_depth_raw, np.float32)
    depth = (np.logaddexp(tdr, np.float32(0.0)).astype(np.float32)
             + np.float32(MIN_DISTANCE)).astype(np.float32)
    fr = np.float32(np.asarray(focal_raw).reshape(-1)[0])
    focal = np.float32(np.logaddexp(fr, np.float32(0.0))) + np.float32(MIN_FOCAL)
    txy = np.asarray(translation_xy, np.float32)
    cxf = np.float32(cx)
    cyf = np.float32(cy)

    Ca = -focal * rot[:, 0, :] + cxf * rot[:, 2, :]      # [V,3]
    sA = -focal * txy[:, 0] - cxf * depth                # [V]
    Cb = focal * rot[:, 1, :] + cyf * rot[:, 2, :]
    sB = focal * txy[:, 1] - cyf * depth
    Cz = rot[:, 2, :]
    sZ = -depth
    return (Ca, sA), (Cb, sB), (Cz, sZ)


def _stationary(C, sbias, dtype):
    """[7, 128] block stationary: col 64g+v <- coeffs of view v, rows 3g+r."""
    st = np.zeros((7, 128), np.float32)
    for g in range(2):
        cols = slice(64 * g, 64 * g + 64)
        for r in range(3):
            st[3 * g + r, cols] = C[:, r]
        st[6, cols] = sbias
    return np.ascontiguousarray(st.astype(dtype))


def kernel(points, euler, translation_xy, translation_depth_raw, focal_raw,
           cx, cy, _trace=False):
    _setup_paths()
    from concourse.bass_utils import run_bass_kernel_spmd

    if "nc" not in _CACHE:
        _CACHE["nc"] = _build_nc()
    nc = _CACHE["nc"]

    points = np.ascontiguousarray(np.asarray(points, np.float32))
    (Ca, sA), (Cb, sB), (Cz, sZ) = _host_precompute(
        euler, translation_xy, translation_depth_raw, focal_raw, cx, cy)

    sta = np.concatenate(
        [_stationary(Ca, sA, np.float16), _stationary(Cb, sB, np.float16)],
        axis=1)  # [7, 256]
    stz = _stationary(Cz, sZ, np.float32)  # [7, 128]

    in_maps = []
    for k in range(NC):
        sl = points[k * N_LOC:(k + 1) * N_LOC]  # [62500, 3]
        mov = np.zeros((7, F_PAD), np.float32)
        for g in range(2):
            seg = sl[g * HALF:(g + 1) * HALF]  # [31250, 3]
            mov[3 * g:3 * g + 3, :HALF] = seg.T
        mov[6, :] = 1.0
        mov32 = np.ascontiguousarray(mov)
        mov16 = np.ascontiguousarray(mov.astype(np.float16))
        in_maps.append({"MOV16": mov16, "MOV32": mov32, "STA": sta, "STZ": stz})

    res = run_bass_kernel_spmd(nc, in_maps, list(range(NC)), trace=_trace)
    _CACHE["last_results"] = res

    out = np.empty((V, N, 2), np.float32)
    for k in range(NC):
        o = res.results[k]["OUT"]  # [128, 2*F_PAD]
        for g in range(2):
            seg = o[64 * g:64 * g + 64, :2 * HALF].reshape(64, HALF, 2)
            out[:, k * N_LOC + g * HALF:k * N_LOC + (g + 1) * HALF, :] = seg
    return out


# revision 4
# speedup vs baseline: 1.9243x; 1.2932x over previous
"""Bundle-adjustment projection kernel for 8 Trainium2 NeuronCores.

out[v, n, :] = (u, v) pixel projection of point n under view v
(reference: nn_BundleAdjustmentModel).

Sharding: points N split 8 ways (62500/core); every core computes all 64
views for its slice. On-chip layout: partition p = 64*g + v where g splits
the core's points into 2 halves of 31250 — so every elementwise op runs
128 partitions wide.

The affine work runs on the otherwise-idle PE (tensor engine). Per 512-col
chunk, three bf16 matmuls with block stationaries [39, 128] compute

  a  = (-f*R0 + cx*R2).p + (-f*tx - cx*depth)
  b  = ( f*R1 + cy*R2).p + ( f*ty - cy*depth)
  zc =            R2.p  - depth

zc feeds a pole (clip at |zc| < 1e-4), so plain 16-bit operands are not
accurate enough. Instead both points and coefficients are split 3-way in
bf16 (p = p0+p1+p2, C = C0+C1+C2, ~24 effective mantissa bits) and the
six dominant cross terms are stacked along the matmul K dim — K costs no
cycles (1 col/cycle for any K <= 128), so one bf16 matmul yields a
near-fp32 affine. Moving rows: [p0, p1, p0, p2, p1, p0] x 2 halves (36) +
3 ones rows for a 3-way-split bias. bf16 (not fp16) so no operand ever
goes subnormal.

Tail per chunk:  DVE    rs = reciprocal_approx_fast(zc)   (PSUM->SBUF)
                 GPSIMD rc = clip(rs, +-1e4)              (SBUF->SBUF; no
                        port contention with 1x-mode DVE)
                 DVE    u = a*rc -> uv[:, 0::2], v = b*rc -> uv[:, 1::2]
                 DMA    uv [128, 1024] fp32 -> HBM (4 KiB/partition)

cx/cy are folded into the PE coefficients (u = (a + cx*zc)/zc = a/zc + cx
exactly when unclipped; error <= cx on clipped points ~ 1.6e-4 of scale).
Host does all O(V) coefficient math + O(N) transposes/splits.
"""
import sys
import types

import numpy as np

V = 64
N = 500000
NC = 8  # cores
N_LOC = N // NC  # 62500 points per core
HALF = N_LOC // 2  # 31250 per partition-half
FW = 512  # chunk width (1 PSUM bank)
NCH = (HALF + FW - 1) // FW  # 62 chunks
F_PAD = NCH * FW  # 31744
K = 39  # moving rows: 6 groups x (3 coords x 2 halves) + 3 ones rows
Z_EPS = 1e-4
RS_MAX = 1.0 / Z_EPS
MIN_FOCAL = 50.0
MIN_DISTANCE = 0.25

# term t: sum_t  C[CIDX[t]] . p[PIDX[t]]  (+ 3-way split bias on ones rows)
PIDX = (0, 1, 0, 2, 1, 0)
CIDX = (0, 0, 1, 0, 1, 2)

_CACHE = {}


def _setup_paths():
    if "/opt/trn_rl_repo" not in sys.path:
        sys.path.insert(0, "/opt/trn_rl_repo")
    # the axon trace path imports antenv.axon_hooks; provide a stub if absent
    try:
        import antenv
        if not hasattr(antenv, "axon_hooks"):
            mod = types.ModuleType("antenv.axon_hooks")
            mod._hook = None
            mod.set_axon_ntff_profile_hook = lambda h: setattr(mod, "_hook", h)
            mod.get_axon_ntff_profile_hook = lambda: mod._hook
            sys.modules["antenv.axon_hooks"] = mod
            antenv.axon_hooks = mod
    except ImportError:
        pass


def _build_nc():
    import concourse.bacc as bacc
    import concourse.mybir as mybir
    from concourse import tile

    dt = mybir.dt
    ALU = mybir.AluOpType

    nc = bacc.Bacc("TRN2", target_bir_lowering=False, debug=False)
    MOV = nc.dram_tensor("MOV", [K, F_PAD], dt.bfloat16, kind="ExternalInput")
    ST = nc.dram_tensor("ST", [K, 384], dt.bfloat16, kind="ExternalInput")
    OUT = nc.dram_tensor("OUT", [128, 2 * F_PAD], dt.float32, kind="ExternalOutput")

    with tile.TileContext(nc) as tc:
        with (
            tc.tile_pool(name="cst", bufs=1) as cpool,
            tc.tile_pool(name="wrk", bufs=3) as wp,
            tc.tile_pool(name="ps", bufs=2, space="PSUM") as pp,
        ):
            mov = cpool.tile([K, F_PAD], dt.bfloat16)
            st = cpool.tile([K, 384], dt.bfloat16)
            nc.sync.dma_start(out=mov[:], in_=MOV.ap())
            nc.sync.dma_start(out=st[:], in_=ST.ap())

            for c in range(NCH):
                s = slice(c * FW, (c + 1) * FW)
                pa = pp.tile([128, FW], dt.float32, name="pa", tag="pa")
                pb = pp.tile([128, FW], dt.float32, name="pb", tag="pb")
                pz = pp.tile([128, FW], dt.float32, name="pz", tag="pz")
                nc.tensor.matmul(pz[:], st[:, 256:384], mov[:, s],
                                 start=True, stop=True)
                nc.tensor.matmul(pa[:], st[:, 0:128], mov[:, s],
                                 start=True, stop=True)
                nc.tensor.matmul(pb[:], st[:, 128:256], mov[:, s],
                                 start=True, stop=True)
                rs = wp.tile([128, FW], dt.float32, name="rs", tag="rs")
                rc = wp.tile([128, FW], dt.float32, name="rc", tag="rc")
                uv = wp.tile([128, 2 * FW], dt.float32, name="uv", tag="uv")
                nc.vector.reciprocal_approx_fast(out=rs[:], in_=pz[:])
                nc.gpsimd.tensor_scalar(rc[:], rs[:], RS_MAX, -RS_MAX,
                                        ALU.min, ALU.max)
                uvv = uv.rearrange("p (n two) -> p two n", two=2)
                nc.vector.tensor_tensor(uvv[:, 0, :], pa[:], rc[:], ALU.mult)
                nc.vector.tensor_tensor(uvv[:, 1, :], pb[:], rc[:], ALU.mult)
                nc.sync.dma_start(out=OUT.ap()[:, 2 * c * FW:2 * (c + 1) * FW],
                                  in_=uv)
    nc.compile()
    return nc


def _host_precompute(euler, translation_xy, translation_depth_raw, focal_raw,
                     cx, cy):
    """Per-view coefficient rows (fp32): (Ca, sA), (Cb, sB), (Cz, sZ)."""
    euler = np.asarray(euler, np.float32)
    c = np.cos(euler)
    s = np.sin(euler)
    cx_, cy_, cz_ = c[:, 0], c[:, 1], c[:, 2]
    sx_, sy_, sz_ = s[:, 0], s[:, 1], s[:, 2]
    one = np.ones_like(cx_)
    zero = np.zeros_like(cx_)
    rx = np.stack([
        np.stack([one, zero, zero], -1),
        np.stack([zero, cx_, -sx_], -1),
        np.stack([zero, sx_, cx_], -1)], -2).astype(np.float32)
    ry = np.stack([
        np.stack([cy_, zero, sy_], -1),
        np.stack([zero, one, zero], -1),
        np.stack([-sy_, zero, cy_], -1)], -2).astype(np.float32)
    rz = np.stack([
        np.stack([cz_, -sz_, zero], -1),
        np.stack([sz_, cz_, zero], -1),
        np.stack([zero, zero, one], -1)], -2).astype(np.float32)
    rot = np.matmul(np.matmul(rx, ry), rz).astype(np.float32)  # [V,3,3]

    tdr = np.asarray(translation_depth_raw, np.float32)
    depth = (np.logaddexp(tdr, np.float32(0.0)).astype(np.float32)
             + np.float32(MIN_DISTANCE)).astype(np.float32)
    fr = np.float32(np.asarray(focal_raw).reshape(-1)[0])
    focal = np.float32(np.logaddexp(fr, np.float32(0.0))) + np.float32(MIN_FOCAL)
    txy = np.asarray(translation_xy, np.float32)
    cxf = np.float32(cx)
    cyf = np.float32(cy)

    Ca = -focal * rot[:, 0, :] + cxf * rot[:, 2, :]      # [V,3]
    sA = -focal * txy[:, 0] - cxf * depth                # [V]
    Cb = focal * rot[:, 1, :] + cyf * rot[:, 2, :]
    sB = focal * txy[:, 1] - cyf * depth
    Cz = rot[:, 2, :]
    sZ = -depth
    return (Ca, sA), (Cb, sB), (Cz, sZ)


def _split3(x):
    """3-way bf16 split: x ~ s[0]+s[1]+s[2], each bf16 (as float32)."""
    import ml_dtypes
    x = np.asarray(x, np.float32)
    out = []
    for _ in range(3):
        h = x.astype(ml_dtypes.bfloat16).astype(np.float32)
        out.append(h)
        x = x - h
    return out


def _stationary(C, sbias):
    """[K, 128] fp32 block stationary for one output type."""
    Cs = _split3(C)        # each [V,3]
    ss = _split3(sbias)    # each [V]
    st = np.zeros((K, 128), np.float32)
    for t in range(6):
        Ct = Cs[CIDX[t]]
        for g in range(2):
            cols = slice(64 * g, 64 * g + 64)
            for r in range(3):
                st[6 * t + 3 * g + r, cols] = Ct[:, r]
    for j in range(3):
        st[36 + j, 0:64] = ss[j]
        st[36 + j, 64:128] = ss[j]
    return st


def _moving(sl):
    """[K, F_PAD] fp32 moving block for one core's point slice [62500, 3]."""
    mov = np.zeros((K, F_PAD), np.float32)
    ps = _split3(sl)  # p0, p1, p2 each [62500, 3]
    for t in range(6):
        pt = ps[PIDX[t]]
        for g in range(2):
            seg = pt[g * HALF:(g + 1) * HALF]  # [31250, 3]
            mov[6 * t + 3 * g:6 * t + 3 * g + 3, :HALF] = seg.T
    mov[36:39, :] = 1.0
    return mov


def kernel(points, euler, translation_xy, translation_depth_raw, focal_raw,
           cx, cy, _trace=False):
    _setup_paths()
    import ml_dtypes
    from concourse.bass_utils import run_bass_kernel_spmd

    if "nc" not in _CACHE:
        _CACHE["nc"] = _build_nc()
    nc = _CACHE["nc"]

    points = np.ascontiguousarray(np.asarray(points, np.float32))
    (Ca, sA), (Cb, sB), (Cz, sZ) = _host_precompute(
        euler, translation_xy, translation_depth_raw, focal_raw, cx, cy)

    st = np.concatenate(
        [_stationary(Ca, sA), _stationary(Cb, sB), _stationary(Cz, sZ)],
        axis=1)  # [K, 384]
    st16 = np.ascontiguousarray(st.astype(ml_dtypes.bfloat16))

    in_maps = []
    for k in range(NC):
        sl = points[k * N_LOC:(k + 1) * N_LOC]  # [62500, 3]
        mov16 = np.ascontiguousarray(_moving(sl).astype(ml_dtypes.bfloat16))
        in_maps.append({"MOV": mov16, "ST": st16})

    res = run_bass_kernel_spmd(nc, in_maps, list(range(NC)), trace=_trace)
    _CACHE["last_results"] = res

    out = np.empty((V, N, 2), np.float32)
    for k in range(NC):
        o = res.results[k]["OUT"]  # [128, 2*F_PAD]
        for g in range(2):
            seg = o[64 * g:64 * g + 64, :2 * HALF].reshape(64, HALF, 2)
            out[:, k * N_LOC + g * HALF:k * N_LOC + (g + 1) * HALF, :] = seg
    return out
